# revision 10
# baseline (speedup 1.0000x reference)
"""3-layer GAT (GATNet) on 8 Trainium2 NeuronCores via Bass/Tile.

Sharding: nodes (and their incident edges, grouped by destination) are
partitioned across the 8 cores; weights are replicated. Per layer:
  dense:  H_ext = A @ W_ext for this core's node shard.  W_ext carries
          extra columns so the same matmul also produces the attention
          scores s_src/s_dst per node.  Rows are written to a gather
          table as [h | 1.0 | s_src | pad] (row bytes % 256 == 0).
  AllGather of the gather table across the 8 cores.
  agg:    per 128-destination block, dma_gather the source-node rows of
          the block's edges (two gathers because dma_gather indices are
          int16: table split at row 32768), fetch s_dst per edge with a
          small indirect DMA from the core-local s_dst array, compute
          w = exp(leaky_relu(s_src+s_dst)), build per-tile selection
          matrices Se_w[j,d] = (dst_local[j]==d)*w[j] with one fused DVE
          tensor_scalar against an iota row, and accumulate
          PSUM[d, :] += Se_w^T @ G on the PE.  The constant-1 column in
          each row makes the same matmuls produce the softmax
          denominators.  Epilogue: divide, add bias, leaky_relu, and
          PE-transpose into the next layer's lhsT chunks (kept in SBUF).
Layer 3 aggregates only for the output nodes (first node of each graph,
batch = arange // graph_size), so it is nearly free.
"""

import os
import sys

import numpy as np

sys.path.insert(0, "/opt/trn_rl_repo")

from contextlib import ExitStack  # noqa: E402

from concourse import bacc, bass, mybir, tile  # noqa: E402
from concourse.bass_utils import run_bass_kernel_spmd  # noqa: E402
from concourse.masks import make_identity  # noqa: E402

F32 = mybir.dt.float32
BF16 = mybir.dt.bfloat16
I16 = mybir.dt.int16
I32 = mybir.dt.int32

P = 128
NCORES = 8
NEG_ATT = 0.2
NEG_ACT = 0.01


def _ceil(a, b):
    return -(-a // b)


class Cfg:
    def __init__(self, n_nodes=50000, in_ch=128, hid=256, out_ch=16, graph=50,
                 use_bf16_tab=False):
        assert in_ch == 128
        self.n_nodes = n_nodes
        self.in_ch = in_ch
        self.hid = hid
        self.out_ch = out_ch
        self.graph = graph
        self.use_bf16_tab = use_bf16_tab
        self.nb = _ceil(_ceil(n_nodes, NCORES), P)   # dst blocks per core
        self.rows = self.nb * P                      # padded rows per core
        self.ntot = self.rows * NCORES
        # gather-table row length in elements; row bytes must be % 256 == 0
        self.row12 = 384 if use_bf16_tab else 320  # 768 B / 1280 B
        self.row3 = 64            # f32: 256 B
        self.tdt = BF16 if use_bf16_tab else F32
        # dma_gather indices are int16; split the gather table so both
        # halves stay addressable
        self.split = min(32768, ((self.ntot // 2) // P) * P)
        assert self.split <= 32767 + 1 and self.ntot - self.split <= 32768


# ----------------------------------------------------------------------------
# host-side edge preprocessing
# ----------------------------------------------------------------------------

def preprocess(edge_src, edge_dst, cfg: Cfg):
    """Build the per-core edge-stream arrays.

    Edge slot layout per (core, dst-block): region A (src < SPLIT) slots
    [0, TA*128), region B slots [TA*128, (TA+TB)*128).  Slot s maps to
    gathered-tile position (partition s%128, tile s//128).  Padding slots
    use gather index 0 (a valid row) and dst_local 999 (never matches)."""
    src = np.asarray(edge_src).astype(np.int64)
    dst = np.asarray(edge_dst).astype(np.int64)
    R, NB = cfg.rows, cfg.nb
    N = cfg.n_nodes

    core = dst // R
    blk = (dst - core * R) // P
    SPLIT = cfg.split
    reg = (src >= SPLIT).astype(np.int64)

    # ---- layers 1/2 stream: group edges by (core, block, region) ----
    key = (core * NB + blk) * 2 + reg
    order = np.argsort(key, kind="stable")
    ks, ss, ds = key[order], src[order], dst[order]
    starts = np.searchsorted(ks, np.arange(2 * NCORES * NB))
    pos = np.arange(len(ks)) - starts[ks]

    nA = np.zeros((NCORES, NB), np.int64)
    nB = np.zeros((NCORES, NB), np.int64)
    np.add.at(nA, (core, blk), 1 - reg)
    np.add.at(nB, (core, blk), reg)
    TA = max(1, _ceil(int(nA.max()), P))
    TB = max(1, _ceil(int(nB.max()), P))
    T = TA + TB

    c_s = ks // (2 * NB)
    b_s = (ks // 2) % NB
    r_s = ks % 2
    slot = np.where(r_s == 0, pos, TA * P + pos)
    p_s = slot % P
    t_s = slot // P

    gidx16 = np.zeros((NCORES, NB, 16, T * 8), np.int16)
    dloc16 = np.zeros((NCORES, NB, 16, T * 8), np.int16)
    dstcol = np.full((NCORES, NB, P, T), 999.0, np.float32)

    wcol = np.where(r_s == 0, pos // 16, TA * 8 + pos // 16)
    val = np.where(r_s == 0, ss, ss - SPLIT).astype(np.int16)
    gidx16[c_s, b_s, pos % 16, wcol] = val

    rloc = ds - c_s * R
    # slot order for the s_dst gather uses the same wrapped layout, but
    # slot index here runs over the full T*128 slot space
    wcol_d = np.where(r_s == 0, slot // 16, slot // 16)
    dloc16[c_s, b_s, slot % 16, wcol_d] = rloc.astype(np.int16)
    dstcol[c_s, b_s, p_s, t_s] = (rloc % P).astype(np.float32)

    gidx = np.tile(gidx16, (1, 1, 8, 1))  # replicate across the 8 Q7 groups
    dloc = np.tile(dloc16, (1, 1, 8, 1))

    # ---- layer-3 stream: only edges into the zero-mask (output) nodes ----
    nodes = np.arange(N)
    zmask_node = (nodes % cfg.graph) == 0
    zcounts = np.bincount(nodes[zmask_node] // R, minlength=NCORES)
    zslot_of = np.full(N, -1, np.int64)
    for c in range(NCORES):
        zn = nodes[zmask_node & (nodes // R == c)]
        zslot_of[zn] = np.arange(len(zn))

    sel = zmask_node[dst]
    s3, d3 = src[sel], dst[sel]
    c3 = d3 // R
    r3 = (s3 >= SPLIT).astype(np.int64)
    key3 = c3 * 2 + r3
    o3 = np.argsort(key3, kind="stable")
    k3, s3, d3 = key3[o3], s3[o3], d3[o3]
    starts3 = np.searchsorted(k3, np.arange(2 * NCORES))
    pos3 = np.arange(len(k3)) - starts3[k3]
    n3A = np.zeros(NCORES, np.int64)
    n3B = np.zeros(NCORES, np.int64)
    np.add.at(n3A, c3, 1 - r3)
    np.add.at(n3B, c3, r3)
    T3A = max(1, _ceil(int(n3A.max()), P))
    T3B = max(1, _ceil(int(n3B.max()), P))
    T3 = T3A + T3B

    cc3 = k3 // 2
    rr3 = k3 % 2
    slot3 = np.where(rr3 == 0, pos3, T3A * P + pos3)
    p3 = slot3 % P
    t3 = slot3 // P
    gidx316 = np.zeros((NCORES, 16, T3 * 8), np.int16)
    dloc316 = np.zeros((NCORES, 16, T3 * 8), np.int16)
    dstcol3 = np.full((NCORES, P, T3), 999.0, np.float32)
    wcol3 = np.where(rr3 == 0, pos3 // 16, T3A * 8 + pos3 // 16)
    val3 = np.where(rr3 == 0, s3, s3 - SPLIT).astype(np.int16)
    gidx316[cc3, pos3 % 16, wcol3] = val3
    rl3 = d3 - cc3 * R
    dloc316[cc3, slot3 % 16, slot3 // 16] = rl3.astype(np.int16)
    dstcol3[cc3, p3, t3] = zslot_of[d3].astype(np.float32)
    gidx3 = np.tile(gidx316, (1, 8, 1))
    dloc3 = np.tile(dloc316, (1, 8, 1))

    return dict(TA=TA, TB=TB, T3A=T3A, T3B=T3B,
                gidx=gidx, dloc=dloc, dstcol=dstcol,
                gidx3=gidx3, dloc3=dloc3, dstcol3=dstcol3,
                zcounts=zcounts)


# ----------------------------------------------------------------------------
# program builder
# ----------------------------------------------------------------------------

def build_program(cfg: Cfg, TA, TB, T3A, T3B):
    NB, R = cfg.nb, cfg.rows
    T, T3 = TA + TB, T3A + T3B
    ROW, ROW3 = cfg.row12, cfg.row3
    tdt = cfg.tdt
    HID, OUT = cfg.hid, cfg.out_ch
    NTOT = cfg.ntot
    SPLIT = cfg.split

    nc = bacc.Bacc("TRN2", target_bir_lowering=False, debug=False,
                   num_devices=NCORES)

    # ---- I/O ----
    xT = nc.dram_tensor("xT", [P, R], F32, kind="ExternalInput")
    W1e = nc.dram_tensor("W1e", [P, HID + 3], F32, kind="ExternalInput")
    W2e = nc.dram_tensor("W2e", [HID, HID + 3], F32, kind="ExternalInput")
    W3e = nc.dram_tensor("W3e", [HID, OUT + 3], F32, kind="ExternalInput")
    b1 = nc.dram_tensor("b1", [1, HID], F32, kind="ExternalInput")
    b2 = nc.dram_tensor("b2", [1, HID], F32, kind="ExternalInput")
    b3 = nc.dram_tensor("b3", [1, OUT], F32, kind="ExternalInput")
    gidx = nc.dram_tensor("gidx", [NB, P, T * 8], I16, kind="ExternalInput")
    dloc = nc.dram_tensor("dloc", [NB, P, T * 8], I16, kind="ExternalInput")
    dstcol = nc.dram_tensor("dstcol", [NB, P, T], F32, kind="ExternalInput")
    gidx3 = nc.dram_tensor("gidx3", [P, T3 * 8], I16, kind="ExternalInput")
    dloc3 = nc.dram_tensor("dloc3", [P, T3 * 8], I16, kind="ExternalInput")
    dstcol3 = nc.dram_tensor("dstcol3", [P, T3], F32, kind="ExternalInput")
    out_d = nc.dram_tensor("out", [P, OUT], F32, kind="ExternalOutput")

    # ---- internal DRAM ----
    h1shard = nc.dram_tensor("h1shard", [R, ROW], tdt)
    h1tab = nc.dram_tensor("h1tab", [NTOT, ROW], tdt, addr_space="Shared")
    h2shard = nc.dram_tensor("h2shard", [R, ROW], tdt)
    h2tab = nc.dram_tensor("h2tab", [NTOT, ROW], tdt, addr_space="Shared")
    h3shard = nc.dram_tensor("h3shard", [R, ROW3], F32)
    h3tab = nc.dram_tensor("h3tab", [NTOT, ROW3], F32, addr_space="Shared")
    SROW = 64
    stab = [nc.dram_tensor(f"stab{i}", [R, SROW], F32) for i in (1, 2, 3)]

    rg = [list(range(NCORES))]

    # persistent next-layer lhsT chunks (A^T), reused across layer pairs
    aT = [nc.alloc_sbuf_tensor("aT0", [P, R], F32),
          nc.alloc_sbuf_tensor("aT1", [P, R], F32)]

    with tile.TileContext(nc) as tc, ExitStack() as ctx:
        cpool = ctx.enter_context(tc.tile_pool(name="const", bufs=1))
        wpool = ctx.enter_context(tc.tile_pool(name="weights", bufs=1))
        lt_pool = ctx.enter_context(tc.tile_pool(name="lhsT", bufs=3))
        row_pool = ctx.enter_context(tc.tile_pool(name="rows", bufs=3))
        idx_pool = ctx.enter_context(tc.tile_pool(name="idx", bufs=3))
        g_pool = ctx.enter_context(tc.tile_pool(name="gather", bufs=2))
        s_pool = ctx.enter_context(tc.tile_pool(name="scal", bufs=4))
        se_pool = ctx.enter_context(tc.tile_pool(name="sew", bufs=4))
        a_pool = ctx.enter_context(tc.tile_pool(name="arow", bufs=3))
        ps_dense = ctx.enter_context(
            tc.tile_pool(name="psd", bufs=2, space="PSUM"))
        ps_agg = ctx.enter_context(
            tc.tile_pool(name="psa", bufs=2, space="PSUM"))
        ps_tp = ctx.enter_context(
            tc.tile_pool(name="pst", bufs=2, space="PSUM"))
        ps_bc = ctx.enter_context(
            tc.tile_pool(name="psb", bufs=1, space="PSUM"))

        # constants
        ident = cpool.tile([P, P], F32, tag="ident")
        make_identity(nc, ident[:])
        iota_i = cpool.tile([P, P], I32, tag="iotai")
        nc.gpsimd.iota(iota_i[:], pattern=[[1, P]], base=0,
                       channel_multiplier=0)
        iota_f = cpool.tile([P, P], F32, tag="iotaf")
        nc.vector.tensor_copy(out=iota_f[:], in_=iota_i[:])
        ones1 = cpool.tile([1, P], F32, tag="ones1")
        nc.vector.memset(ones1[:], 1.0)

        # preload weights
        w1_sb = wpool.tile([P, HID + 3], F32, tag="w1")
        nc.sync.dma_start(out=w1_sb[:], in_=W1e[:, :])
        w2_sb = [wpool.tile([P, HID + 3], F32, tag=f"w2_{k}",
                            name=f"w2sb{k}") for k in range(2)]
        for k in range(2):
            nc.sync.dma_start(out=w2_sb[k][:], in_=W2e[k * P:(k + 1) * P, :])
        w3_sb = [wpool.tile([P, OUT + 3], F32, tag=f"w3_{k}",
                            name=f"w3sb{k}") for k in range(2)]
        for k in range(2):
            nc.sync.dma_start(out=w3_sb[k][:], in_=W3e[k * P:(k + 1) * P, :])

        def bias_bcast(bd, C, tag):
            brow = cpool.tile([1, C], F32, tag=f"brow_{tag}")
            nc.sync.dma_start(out=brow[:], in_=bd[:, :])
            bps = ps_bc.tile([P, C], F32, tag="bps")
            nc.tensor.matmul(bps[:], lhsT=ones1[:], rhs=brow[:],
                             start=True, stop=True)
            bbc = cpool.tile([P, C], F32, tag=f"bbc_{tag}")
            nc.vector.tensor_copy(out=bbc[:], in_=bps[:])
            return bbc

        # ------------------------------------------------------------------
        def dense_phase(layer, w_tiles, shard, sdacc_dst, rowlen, row_dt,
                        one_col, sd_col):
            """H_ext = A @ W_ext for this core's rows; writes the gather
            table rows [h | 1 | s_src | pad] + the local s_dst array."""
            n_extcols = sd_col + 1
            for it in range(NB):
                if layer == 1:
                    lt = lt_pool.tile([P, P], F32, tag="xT")
                    nc.sync.dma_start(out=lt[:],
                                      in_=xT[:, it * P:(it + 1) * P])
                    lts = [lt[:]]
                else:
                    lts = [aT[k].ap()[:, it * P:(it + 1) * P]
                           for k in range(2)]
                ps = ps_dense.tile([P, n_extcols], F32, tag="dps")
                for k, lt_ap in enumerate(lts):
                    nc.tensor.matmul(ps[:], lhsT=lt_ap, rhs=w_tiles[k][:],
                                     start=(k == 0), stop=(k == len(lts) - 1))
                row = row_pool.tile([P, rowlen], row_dt, tag=f"row{rowlen}")
                if row_dt == BF16:
                    # [h bf16 | 1.0 | s_src_hi | s_src_lo | pad]
                    nc.vector.tensor_copy(out=row[:, 0:one_col],
                                          in_=ps[:, 0:one_col])
                    nc.vector.memset(row[:, one_col:rowlen], 0.0)
                    nc.vector.memset(row[:, one_col:one_col + 1], 1.0)
                    hc = one_col + 1
                    nc.vector.tensor_copy(out=row[:, hc:hc + 1],
                                          in_=ps[:, hc:hc + 1])
                    shi = s_pool.tile([P, 1], F32, tag="shi")
                    nc.vector.tensor_copy(out=shi[:], in_=row[:, hc:hc + 1])
                    slo = s_pool.tile([P, 1], F32, tag="slo")
                    nc.vector.tensor_tensor(out=slo[:], in0=ps[:, hc:hc + 1],
                                            in1=shi[:],
                                            op=mybir.AluOpType.subtract)
                    nc.vector.tensor_copy(out=row[:, hc + 1:hc + 2],
                                          in_=slo[:])
                else:
                    # psum col one_col is 0 (zero column in W_ext);
                    # col one_col+1 is s_src
                    nc.vector.tensor_copy(out=row[:, 0:one_col + 2],
                                          in_=ps[:, 0:one_col + 2])
                    nc.vector.memset(row[:, one_col:one_col + 1], 1.0)
                    nc.vector.memset(row[:, one_col + 2:rowlen], 0.0)
                srow = row_pool.tile([P, SROW], F32, tag="srow")
                nc.vector.memset(srow[:, 1:SROW], 0.0)
                nc.vector.tensor_copy(out=srow[:, 0:1],
                                      in_=ps[:, sd_col:sd_col + 1])
                nc.sync.dma_start(out=shard[it * P:(it + 1) * P, :],
                                  in_=row[:])
                nc.sync.dma_start(
                    out=sdacc_dst[it * P:(it + 1) * P, :], in_=srow[:])

        # ------------------------------------------------------------------
        def agg_phase(layer, tab, sd_t, nblocks, tA, tB, rowlen, row_dt,
                      gidx_t, dloc_t, dstcol_t, C_out, bbc):
            tT = tA + tB
            n_mm = C_out + 1  # h columns + the constant-1 (denominator) col
            tabA = tab[0:SPLIT, :]
            tabB = tab[SPLIT:NTOT, :]
            for b in range(nblocks):
                gi = idx_pool.tile([P, tT * 8], I16, tag="gi")
                dl = idx_pool.tile([P, tT * 8], I16, tag="dl")
                dc = idx_pool.tile([P, tT], F32, tag="dc")
                if nblocks == 1:
                    nc.sync.dma_start(out=gi[:], in_=gidx_t[:, :])
                    nc.sync.dma_start(out=dl[:], in_=dloc_t[:, :])
                    nc.sync.dma_start(out=dc[:], in_=dstcol_t[:, :])
                else:
                    nc.sync.dma_start(out=gi[:], in_=gidx_t[b, :, :])
                    nc.sync.dma_start(out=dl[:], in_=dloc_t[b, :, :])
                    nc.sync.dma_start(out=dc[:], in_=dstcol_t[b, :, :])
                G = g_pool.tile([P, tT * rowlen], row_dt, tag=f"G{rowlen}")
                G3d = G[:].rearrange("p (t c) -> p t c", c=rowlen)
                nc.gpsimd.dma_gather(
                    out_ap=G3d[:, 0:tA, :], in_ap=tabA,
                    idxs_ap=gi[:, 0:tA * 8], num_idxs=tA * P,
                    num_idxs_reg=tA * P, elem_size=rowlen, elem_step=rowlen,
                    single_packet=False)
                nc.gpsimd.dma_gather(
                    out_ap=G3d[:, tA:tT, :], in_ap=tabB,
                    idxs_ap=gi[:, tA * 8:tT * 8], num_idxs=tB * P,
                    num_idxs_reg=tB * P, elem_size=rowlen, elem_step=rowlen,
                    single_packet=False)
                Gs = g_pool.tile([P, tT * SROW], F32, tag="Gs")
                Gs3d = Gs[:].rearrange("p (t c) -> p t c", c=SROW)
                nc.gpsimd.dma_gather(
                    out_ap=Gs3d[:, :, :], in_ap=sd_t[:, :],
                    idxs_ap=dl[:], num_idxs=tT * P,
                    num_idxs_reg=tT * P, elem_size=SROW, elem_step=SROW,
                    single_packet=False)
                sdp = Gs[:, 0::SROW]
                if row_dt == BF16:
                    sfull = s_pool.tile([P, tT], F32, tag="sfull")
                    nc.vector.tensor_tensor(
                        out=sfull[:], in0=G[:, C_out + 1::rowlen],
                        in1=G[:, C_out + 2::rowlen], op=mybir.AluOpType.add)
                    ssrc = sfull[:]
                else:
                    ssrc = G[:, C_out + 1::rowlen]
                z = s_pool.tile([P, tT], F32, tag="z")
                nc.vector.tensor_tensor(out=z[:], in0=ssrc, in1=sdp,
                                        op=mybir.AluOpType.add)
                e = s_pool.tile([P, tT], F32, tag="e")
                nc.vector.scalar_tensor_tensor(
                    out=e[:], in0=z[:], scalar=NEG_ATT, in1=z[:],
                    op0=mybir.AluOpType.mult, op1=mybir.AluOpType.max)
                w = s_pool.tile([P, tT], F32, tag="w")
                nc.scalar.activation(w[:], e[:],
                                     mybir.ActivationFunctionType.Exp)
                ps = ps_agg.tile([P, n_mm], F32, tag="aps")
                for t in range(tT):
                    sw = se_pool.tile([P, P], row_dt, tag="sw")
                    nc.vector.tensor_scalar(
                        out=sw[:], in0=iota_f[:], scalar1=dc[:, t:t + 1],
                        scalar2=w[:, t:t + 1],
                        op0=mybir.AluOpType.is_equal,
                        op1=mybir.AluOpType.mult)
                    nc.tensor.matmul(
                        ps[:], lhsT=sw[:],
                        rhs=G[:, t * rowlen:t * rowlen + n_mm],
                        start=(t == 0), stop=(t == tT - 1))
                dn = s_pool.tile([P, 1], F32, tag="dn")
                nc.vector.tensor_scalar_add(dn[:], ps[:, C_out:C_out + 1],
                                            1e-30)
                rc = s_pool.tile([P, 1], F32, tag="rc")
                nc.vector.reciprocal(rc[:], dn[:])
                ar = a_pool.tile([P, C_out], F32, tag="ar")
                nc.scalar.activation(ar[:], ps[:, 0:C_out],
                                     mybir.ActivationFunctionType.Copy,
                                     scale=rc[:])
                nc.vector.tensor_tensor(out=ar[:], in0=ar[:], in1=bbc[:],
                                        op=mybir.AluOpType.add)
                if layer < 3:
                    ar2 = a_pool.tile([P, C_out], F32, tag="ar2")
                    nc.vector.scalar_tensor_tensor(
                        out=ar2[:], in0=ar[:], scalar=NEG_ACT, in1=ar[:],
                        op0=mybir.AluOpType.mult, op1=mybir.AluOpType.max)
                    for k in range(2):
                        tp = ps_tp.tile([P, P], F32, tag="tp")
                        nc.tensor.transpose(tp[:], ar2[:, k * P:(k + 1) * P],
                                            ident[:])
                        nc.vector.tensor_copy(
                            out=aT[k].ap()[:, b * P:(b + 1) * P], in_=tp[:])
                else:
                    nc.sync.dma_start(out=out_d[:, :], in_=ar[:])

        # ====================== the network ======================
        bbc1 = bias_bcast(b1, HID, "b1")
        dense_phase(1, [w1_sb], h1shard, stab[0], ROW, tdt,
                    one_col=HID, sd_col=HID + 2)
        nc.gpsimd.collective_compute(
            "AllGather", mybir.AluOpType.bypass, replica_groups=rg,
            ins=[h1shard.ap()], outs=[h1tab.ap()])
        agg_phase(1, h1tab, stab[0], NB, TA, TB, ROW, tdt,
                  gidx, dloc, dstcol, HID, bbc1)

        bbc2 = bias_bcast(b2, HID, "b2")
        dense_phase(2, w2_sb, h2shard, stab[1], ROW, tdt,
                    one_col=HID, sd_col=HID + 2)
        nc.gpsimd.collective_compute(
            "AllGather", mybir.AluOpType.bypass, replica_groups=rg,
            ins=[h2shard.ap()], outs=[h2tab.ap()])
        agg_phase(2, h2tab, stab[1], NB, TA, TB, ROW, tdt,
                  gidx, dloc, dstcol, HID, bbc2)

        bbc3 = bias_bcast(b3, OUT, "b3")
        dense_phase(3, w3_sb, h3shard, stab[2], ROW3, F32,
                    one_col=OUT, sd_col=OUT + 2)
        nc.gpsimd.collective_compute(
            "AllGather", mybir.AluOpType.bypass, replica_groups=rg,
            ins=[h3shard.ap()], outs=[h3tab.ap()])
        agg_phase(3, h3tab, stab[2], 1, T3A, T3B, ROW3, F32,
                  gidx3, dloc3, dstcol3, OUT, bbc3)

    nc.compile()
    return nc


# ----------------------------------------------------------------------------
# host wrapper
# ----------------------------------------------------------------------------

def make_in_maps(inputs, pre, cfg: Cfg):
    R = cfg.rows
    N = cfg.n_nodes
    x = np.asarray(inputs["x"], np.float32)

    def wext(W, a_s, a_d):
        W = np.asarray(W, np.float32)
        a_s = np.asarray(a_s, np.float32)
        a_d = np.asarray(a_d, np.float32)
        z = np.zeros((W.shape[0], 1), np.float32)
        return np.concatenate(
            [W, z, (W @ a_s)[:, None], (W @ a_d)[:, None]], axis=1
        ).astype(np.float32)

    W1e = wext(inputs["W1"], inputs["a_src1"], inputs["a_dst1"])
    W2e = wext(inputs["W2"], inputs["a_src2"], inputs["a_dst2"])
    W3e = wext(inputs["W3"], inputs["a_src3"], inputs["a_dst3"])
    b1 = np.asarray(inputs["b1"], np.float32).reshape(1, -1)
    b2 = np.asarray(inputs["b2"], np.float32).reshape(1, -1)
    b3 = np.asarray(inputs["b3"], np.float32).reshape(1, -1)
    in_maps = []
    for c in range(NCORES):
        lo, hi = c * R, min((c + 1) * R, N)
        xs = np.zeros((P, R), np.float32)
        xs[:, 0:hi - lo] = x[lo:hi].T
        in_maps.append({
            "xT": xs, "W1e": W1e, "W2e": W2e, "W3e": W3e,
            "b1": b1, "b2": b2, "b3": b3,
            "gidx": pre["gidx"][c], "dloc": pre["dloc"][c],
            "dstcol": pre["dstcol"][c],
            "gidx3": pre["gidx3"][c], "dloc3": pre["dloc3"][c],
            "dstcol3": pre["dstcol3"][c],
        })
    return in_maps


_CACHE = {}


def get_program(cfg: Cfg, TA, TB, T3A, T3B):
    key = (cfg.n_nodes, cfg.use_bf16_tab, TA, TB, T3A, T3B)
    if key not in _CACHE:
        _CACHE[key] = build_program(cfg, TA, TB, T3A, T3B)
    return _CACHE[key]


def run(inputs, cfg: Cfg, trace=False):
    pre = preprocess(inputs["edge_src"], inputs["edge_dst"], cfg)
    in_maps = make_in_maps(inputs, pre, cfg)
    nc = get_program(cfg, pre["TA"], pre["TB"], pre["T3A"], pre["T3B"])
    res = run_bass_kernel_spmd(nc, in_maps, list(range(NCORES)), trace=trace)
    outs = []
    for c in range(NCORES):
        outs.append(res.results[c]["out"][0:pre["zcounts"][c], :])
    return np.concatenate(outs, axis=0).astype(np.float32), res


def kernel(**inputs):
    cfg = Cfg(n_nodes=inputs["x"].shape[0],
              in_ch=inputs["x"].shape[1],
              hid=inputs["W1"].shape[1],
              out_ch=inputs["W3"].shape[1],
              use_bf16_tab=os.environ.get("GAT_BF16", "0") == "1")
    out, _ = run(inputs, cfg)
    return out
